# revision 2
# baseline (speedup 1.0000x reference)
# Trainium2 Bass kernel for nn_DirectedChainSDE.
#
# Math (per batch row b, 255 steps):
#   z_s = [x_s (64), nei[b, s] (65)]                       (129)
#   drift = MLP_dr(z)   : 129 -> 512 -> 512 -> 512 -> 64   (lipswish = .909*silu)
#   diffu = MLP_df(z)   : 129 -> 512 -> 512 -> 512 -> 2048 (reshape 64x32)
#   x_{s+1} = x_s + dt*drift + einsum('hn,n->h', diffu, bm[s,b]*sqrt(dt))
#
# Strategy: data-parallel over batch (8 cores x 512 rows). Everything
# feature-major on chip: activations [features(part), batch(free=512)].
# All matmuls float32r (full-rate fp32 storage). Constant folds done on the
# host: 0.909 into W2/W3/W4, dt into drift W4/b4, sqrt(dt) into bm.
# The noise einsum runs as: elementwise (diffu_chunk * dW_rep) on DVE, then a
# 0/1-selector matmul on PE reducing each 32-partition group, accumulating
# straight into the x_next PSUM tile together with the drift matmuls, the
# diffusion-bias matmul (b4df.reshape(64,32).T @ dW) and finally
# x_next = (psum + dt*b4dr) + x_s on DVE.

import numpy as np

HID = 64
NOISE = 32
MLP = 512
LEN = 256
BATCH = 4096
NCORES = 8
BL = BATCH // NCORES          # 512 rows per core
NSTEPS = LEN - 1              # 255
UNROLL = 3                    # 255 = 3 * 85
LSW = 0.909

_CACHE = {}


def _split_excess_waits(nc, mybir, max_waits=1):
    """Walrus (CoreV3GenImpl setupSyncWait) accepts at most one embedded sync
    wait per instruction; Tile's final drain (and occasionally other insts)
    carries several.  Spill the extras onto same-engine NoOps placed directly
    before the instruction — semantics are identical (sems are monotonic and
    engine execution is sequential)."""
    for f in nc.m.functions:
        for bb in f.blocks:
            new_list = []
            changed = False
            for ins in bb.instructions:
                si = getattr(ins, "sync_info", None)
                waits = list(si.on_wait) if si is not None else []
                if len(waits) > max_waits:
                    changed = True
                    keep = waits[-max_waits:]
                    spill = waits[:-max_waits]
                    for i in range(0, len(spill), max_waits):
                        chunk = spill[i:i + max_waits]
                        new_list.append(mybir.InstNoOp(
                            name=nc.get_next_instruction_name(),
                            engine=ins.engine,
                            ins=[], outs=[],
                            sync_info=mybir.SyncInfo(on_wait=chunk, on_update=[]),
                        ))
                    ins.sync_info = mybir.SyncInfo(
                        on_wait=keep, on_update=list(si.on_update))
                new_list.append(ins)
            if changed:
                bb.instructions = new_list


def _build_nc(nsteps=NSTEPS, unroll=UNROLL):
    import concourse.bass as bass
    import concourse.mybir as mybir
    import concourse.tile as tile
    from concourse.bass import ds

    f32 = mybir.dt.float32
    f32r = mybir.dt.float32r
    SILU = mybir.ActivationFunctionType.Silu
    ADD = mybir.AluOpType.add

    assert nsteps % unroll == 0
    niter = nsteps // unroll

    nc = bass.Bass()

    nei_d = nc.dram_tensor("nei", [nsteps, 65, BL], f32r, kind="ExternalInput")
    dw4_d = nc.dram_tensor("dw4", [nsteps, 128, BL], f32r, kind="ExternalInput")
    x0_d = nc.dram_tensor("x0", [HID, BL], f32r, kind="ExternalInput")
    out_d = nc.dram_tensor("out", [nsteps, HID, BL], f32, kind="ExternalOutput")

    wts = {}
    for pfx in ("dr", "df"):
        wts[pfx + "1x"] = nc.dram_tensor("w1x_" + pfx, [64, 512], f32r, kind="ExternalInput")
        wts[pfx + "1n"] = nc.dram_tensor("w1n_" + pfx, [65, 512], f32r, kind="ExternalInput")
        wts[pfx + "2"] = nc.dram_tensor("w2_" + pfx, [128, 2048], f32r, kind="ExternalInput")
        wts[pfx + "3"] = nc.dram_tensor("w3_" + pfx, [128, 2048], f32r, kind="ExternalInput")
    wts["dr4"] = nc.dram_tensor("w4_dr", [128, 4 * HID], f32r, kind="ExternalInput")
    wts["df4"] = nc.dram_tensor("w4_df", [128, 4 * 2048], f32r, kind="ExternalInput")
    bias_d = nc.dram_tensor("biases", [128, 25], f32, kind="ExternalInput")
    b4t_d = nc.dram_tensor("b4dfT", [32, 64], f32r, kind="ExternalInput")
    tsel_d = nc.dram_tensor("tsel", [128, 124], f32r, kind="ExternalInput")

    def r(ap):
        return ap

    with tile.TileContext(nc) as tc:
        with (
            tc.tile_pool(name="wpool", bufs=1) as wpool,
            tc.tile_pool(name="xpool", bufs=1) as xpool,
            tc.tile_pool(name="acts", bufs=2) as acts,
            tc.tile_pool(name="inp", bufs=3) as inp,
            tc.tile_pool(name="tmp", bufs=3) as tmpp,
            tc.tile_pool(name="lay", bufs=2, space="PSUM") as lay,
            tc.tile_pool(name="l4", bufs=2, space="PSUM") as l4p,
            tc.tile_pool(name="accp", bufs=2, space="PSUM") as accp,
        ):
            # ---- persistent constants ----
            wsb = {}
            for key, dram in wts.items():
                t = wpool.tile(list(dram.shape), f32r, tag="w_" + key)
                nc.sync.dma_start(t[:], dram[:])
                wsb[key] = t
            bias_sb = wpool.tile([128, 25], f32, tag="biases")
            nc.sync.dma_start(bias_sb[:], bias_d[:])
            b4t_sb = wpool.tile([32, 64], f32r, tag="b4dfT")
            nc.sync.dma_start(b4t_sb[:], b4t_d[:])
            tsel_sb = wpool.tile([128, 124], f32r, tag="tsel")
            nc.sync.dma_start(tsel_sb[:], tsel_d[:])

            xt = []
            for j in range(unroll):
                t = xpool.tile([HID, BL], f32r, tag=f"x{j}")
                xt.append(t)
            nc.sync.dma_start(xt[0][:], x0_d[:])

            def mlp_3layers(pfx, nei_t, x_in, bias_base):
                """L1..L3 of one MLP; returns h3 tile [128, 4*512]."""
                h_prev = None
                for li, wkey in ((0, pfx + "1x"), ):
                    pass
                # L1
                h1 = acts.tile([128, 2048], f32r, tag="h1")
                w1x, w1n = wsb[pfx + "1x"], wsb[pfx + "1n"]
                for m in range(4):
                    p = lay.tile([128, 512], f32, tag="lay")
                    nc.tensor.matmul(p[:], r(w1x[:, m * 128:(m + 1) * 128]),
                                     r(x_in[:]), start=True, stop=False)
                    nc.tensor.matmul(p[:], r(w1n[:, m * 128:(m + 1) * 128]),
                                     r(nei_t[:]), start=False, stop=True)
                    nc.scalar.activation(h1[:, m * 512:(m + 1) * 512], p[:], SILU,
                                         bias=bias_sb[:, bias_base + m:bias_base + m + 1])
                h_prev = h1
                # L2, L3
                for li, wkey in ((1, pfx + "2"), (2, pfx + "3")):
                    h = acts.tile([128, 2048], f32r, tag=f"h{li + 1}")
                    w = wsb[wkey]
                    for m in range(4):
                        p = lay.tile([128, 512], f32, tag="lay")
                        for k in range(4):
                            nc.tensor.matmul(
                                p[:],
                                r(w[:, k * 512 + m * 128: k * 512 + (m + 1) * 128]),
                                r(h_prev[:, k * 512:(k + 1) * 512]),
                                start=(k == 0), stop=(k == 3))
                        col = bias_base + 4 * li + m
                        nc.scalar.activation(h[:, m * 512:(m + 1) * 512], p[:], SILU,
                                             bias=bias_sb[:, col:col + 1])
                    h_prev = h
                return h_prev

            def step(s_expr, x_in, x_out):
                nei_t = inp.tile([65, BL], f32r, tag="nei")
                nc.sync.dma_start(nei_t[:], nei_d[ds(s_expr, 1)])
                dw4_t = inp.tile([128, BL], f32r, tag="dw4")
                nc.sync.dma_start(dw4_t[:], dw4_d[ds(s_expr, 1)])

                h3dr = mlp_3layers("dr", nei_t, x_in, 0)
                h3df = mlp_3layers("df", nei_t, x_in, 12)

                acc = accp.tile([HID, BL], f32, tag="acc")
                # drift L4 (dt and .909 folded into weights)
                w4dr = wsb["dr4"]
                for k in range(4):
                    nc.tensor.matmul(acc[:],
                                     r(w4dr[:, k * HID:(k + 1) * HID]),
                                     r(h3dr[:, k * 512:(k + 1) * 512]),
                                     start=(k == 0), stop=False)
                # diffusion bias term: b4df.reshape(64,32).T @ dW
                nc.tensor.matmul(acc[:], r(b4t_sb[:]), r(dw4_t[0:32, :]),
                                 start=False, stop=False)
                # diffusion L4 + einsum
                w4df = wsb["df4"]
                for c in range(16):
                    pd = l4p.tile([128, 512], f32, tag="l4")
                    for k in range(4):
                        nc.tensor.matmul(
                            pd[:],
                            r(w4df[:, k * 2048 + c * 128: k * 2048 + (c + 1) * 128]),
                            r(h3df[:, k * 512:(k + 1) * 512]),
                            start=(k == 0), stop=(k == 3))
                    tmp = tmpp.tile([128, 512], f32r, tag="tmp")
                    nc.vector.tensor_mul(tmp[:], pd[:], dw4_t[:].bitcast(f32))
                    nc.tensor.matmul(acc[:],
                                     r(tsel_sb[:, 60 - 4 * c: 124 - 4 * c]),
                                     r(tmp[:]), start=False, stop=(c == 15))
                # x_next = (acc + dt*b4dr) + x_in
                nc.vector.scalar_tensor_tensor(
                    x_out[:], acc[:], bias_sb[0:HID, 24:25],
                    x_in[:].bitcast(f32), op0=ADD, op1=ADD)
                nc.sync.dma_start(out_d[ds(s_expr, 1)], x_out[:].bitcast(f32))

            with tc.For_i(0, nsteps, unroll,
                          hint_engines=(mybir.EngineType.PE,)) as iv:
                for u in range(unroll):
                    step(iv + u, xt[u], xt[(u + 1) % unroll])

    _split_excess_waits(nc, mybir)
    return nc


def _pack_k(W):
    """[K, M] -> [128, (K/128)*M] with K-chunk k at columns [k*M, (k+1)*M)."""
    K, M = W.shape
    assert K % 128 == 0
    return np.concatenate([W[k * 128:(k + 1) * 128] for k in range(K // 128)],
                          axis=1)


def _prep_inputs(ts, nei, initial, bm, drift_params, diffusion_params,
                 nsteps=NSTEPS):
    ts = np.asarray(ts, np.float32)
    nei = np.asarray(nei, np.float32)
    initial = np.asarray(initial, np.float32)
    bm = np.asarray(bm, np.float32)
    dts = (ts[1:] - ts[:-1]).astype(np.float32)
    dt = float(dts.mean())

    dr = [(np.asarray(w, np.float32), np.asarray(b, np.float32))
          for w, b in drift_params]
    df = [(np.asarray(w, np.float32), np.asarray(b, np.float32))
          for w, b in diffusion_params]

    common = {}
    for pfx, params in (("dr", dr), ("df", df)):
        W1 = params[0][0]
        common["w1x_" + pfx] = np.ascontiguousarray(W1[0:64])
        common["w1n_" + pfx] = np.ascontiguousarray(W1[64:129])
        common["w2_" + pfx] = _pack_k(LSW * params[1][0])
        common["w3_" + pfx] = _pack_k(LSW * params[2][0])
    common["w4_dr"] = _pack_k((LSW * dt) * dr[3][0])
    common["w4_df"] = _pack_k(LSW * df[3][0])

    biases = np.zeros((128, 25), np.float32)
    for j, (pfx, params) in enumerate((("dr", dr), ("df", df))):
        for li in range(3):
            b = params[li][1]
            for m in range(4):
                biases[:, 12 * j + 4 * li + m] = b[m * 128:(m + 1) * 128]
    biases[0:HID, 24] = dt * dr[3][1]
    common["biases"] = biases
    common["b4dfT"] = np.ascontiguousarray(df[3][1].reshape(HID, NOISE).T)

    tsel = np.zeros((128, 124), np.float32)
    for p in range(128):
        tsel[p, 60 + p // 32] = 1.0
    common["tsel"] = tsel

    sq = np.sqrt(dts[:nsteps]).astype(np.float32)[:, None, None]
    in_maps = []
    for j in range(NCORES):
        sl = slice(j * BL, (j + 1) * BL)
        m = dict(common)
        m["nei"] = np.ascontiguousarray(
            nei[sl, :nsteps].transpose(1, 2, 0))                  # [S, 65, BL]
        bmt = (bm[:nsteps, sl] * sq).transpose(0, 2, 1)           # [S, 32, BL]
        m["dw4"] = np.ascontiguousarray(np.tile(bmt, (1, 4, 1)))  # [S, 128, BL]
        m["x0"] = np.ascontiguousarray(initial[sl].T)             # [64, BL]
        in_maps.append(m)
    return in_maps


def _run(inputs, trace=False, nsteps=NSTEPS, unroll=UNROLL):
    from concourse.bass_utils import run_bass_kernel_spmd

    key = (nsteps, unroll)
    if key not in _CACHE:
        _CACHE[key] = _build_nc(nsteps, unroll)
    nc = _CACHE[key]

    in_maps = _prep_inputs(
        inputs["ts"], inputs["nei"], inputs["initial"], inputs["bm"],
        inputs["drift_params"], inputs["diffusion_params"], nsteps)

    res = run_bass_kernel_spmd(nc, in_maps, core_ids=list(range(NCORES)),
                               trace=trace)

    ts = np.asarray(inputs["ts"], np.float32)
    initial = np.asarray(inputs["initial"], np.float32)
    L = ts.shape[0]
    out = np.empty((BATCH, L, 1 + HID), np.float32)
    out[:, :, 0] = ts[None, :]
    out[:, 0, 1:] = initial
    for j in range(NCORES):
        sl = slice(j * BL, (j + 1) * BL)
        xs = res.results[j]["out"]                     # [S, 64, BL]
        out[sl, 1:nsteps + 1, 1:] = xs.transpose(2, 0, 1)
    return out, res


def kernel(ts, nei, initial, bm, drift_params, diffusion_params,
           batch_size=None):
    out, _ = _run(dict(ts=ts, nei=nei, initial=initial, bm=bm,
                       drift_params=drift_params,
                       diffusion_params=diffusion_params))
    return out


# revision 9
# speedup vs baseline: 1.1905x; 1.1905x over previous
# Trainium2 Bass kernel for nn_DirectedChainSDE.
#
# Math (per batch row b, 255 steps):
#   z_s = [x_s (64), nei[b, s] (65)]                       (129)
#   drift = MLP_dr(z)   : 129 -> 512 -> 512 -> 512 -> 64   (lipswish = .909*silu)
#   diffu = MLP_df(z)   : 129 -> 512 -> 512 -> 512 -> 2048 (reshape 64x32)
#   x_{s+1} = x_s + dt*drift + einsum('hn,n->h', diffu, bm[s,b]*sqrt(dt))
#
# Strategy: data-parallel over batch (8 cores x 512 rows). Everything
# feature-major on chip: activations [features(part), batch(free=512)].
# All matmuls float32r (full-rate fp32 storage). Constant folds done on the
# host: 0.909 into W2/W3/W4, dt into drift W4/b4, sqrt(dt) into bm.
# The noise einsum runs as: elementwise (diffu_chunk * dW_rep) on DVE, then a
# 0/1-selector matmul on PE reducing each 32-partition group, accumulating
# straight into the x_next PSUM tile together with the drift matmuls, the
# diffusion-bias matmul (b4df.reshape(64,32).T @ dW) and finally
# x_next = (psum + dt*b4dr) + x_s on DVE.

import numpy as np

HID = 64
NOISE = 32
MLP = 512
LEN = 256
BATCH = 4096
NCORES = 8
BL = BATCH // NCORES          # 512 rows per core
NSTEPS = LEN - 1              # 255
UNROLL = 5                    # 255 = 5 * 51
LSW = 0.909

_CACHE = {}


def _split_excess_waits(nc, mybir, max_waits=1):
    """Walrus (CoreV3GenImpl setupSyncWait) accepts at most one embedded sync
    wait per instruction; Tile's final drain (and occasionally other insts)
    carries several.  Spill the extras onto same-engine NoOps placed directly
    before the instruction — semantics are identical (sems are monotonic and
    engine execution is sequential)."""
    for f in nc.m.functions:
        for bb in f.blocks:
            new_list = []
            changed = False
            for ins in bb.instructions:
                si = getattr(ins, "sync_info", None)
                waits = list(si.on_wait) if si is not None else []
                if len(waits) > max_waits:
                    changed = True
                    keep = waits[-max_waits:]
                    spill = waits[:-max_waits]
                    for i in range(0, len(spill), max_waits):
                        chunk = spill[i:i + max_waits]
                        new_list.append(mybir.InstNoOp(
                            name=nc.get_next_instruction_name(),
                            engine=ins.engine,
                            ins=[], outs=[],
                            sync_info=mybir.SyncInfo(on_wait=chunk, on_update=[]),
                        ))
                    ins.sync_info = mybir.SyncInfo(
                        on_wait=keep, on_update=list(si.on_update))
                new_list.append(ins)
            if changed:
                bb.instructions = new_list


def _build_nc(nsteps=NSTEPS, unroll=UNROLL):
    import concourse.bass as bass
    import concourse.mybir as mybir
    import concourse.tile as tile
    from concourse.bass import ds

    f32 = mybir.dt.float32
    f32r = mybir.dt.float32r
    SILU = mybir.ActivationFunctionType.Silu
    ADD = mybir.AluOpType.add

    assert nsteps % unroll == 0
    niter = nsteps // unroll

    nc = bass.Bass()

    nei_d = nc.dram_tensor("nei", [nsteps, 65, BL], f32r, kind="ExternalInput")
    dw4_d = nc.dram_tensor("dw4", [nsteps, 128, BL], f32r, kind="ExternalInput")
    x0_d = nc.dram_tensor("x0", [HID, BL], f32r, kind="ExternalInput")
    out_d = nc.dram_tensor("out", [nsteps, HID, BL], f32, kind="ExternalOutput")

    wts = {}
    for pfx in ("dr", "df"):
        wts[pfx + "1x"] = nc.dram_tensor("w1x_" + pfx, [64, 512], f32r, kind="ExternalInput")
        wts[pfx + "1n"] = nc.dram_tensor("w1n_" + pfx, [65, 512], f32r, kind="ExternalInput")
        wts[pfx + "2"] = nc.dram_tensor("w2_" + pfx, [128, 2048], f32r, kind="ExternalInput")
        wts[pfx + "3"] = nc.dram_tensor("w3_" + pfx, [128, 2048], f32r, kind="ExternalInput")
    wts["dr4"] = nc.dram_tensor("w4_dr", [128, 4 * HID], f32r, kind="ExternalInput")
    wts["df4"] = nc.dram_tensor("w4_df", [128, 4 * 2048], f32r, kind="ExternalInput")
    bias_d = nc.dram_tensor("biases", [128, 25], f32, kind="ExternalInput")
    b4t_d = nc.dram_tensor("b4dfT", [32, 64], f32r, kind="ExternalInput")
    tsel_d = nc.dram_tensor("tsel", [128, 124], f32r, kind="ExternalInput")

    def r(ap):
        return ap

    with tile.TileContext(nc) as tc:
        with (
            tc.tile_pool(name="wpool", bufs=1) as wpool,
            tc.tile_pool(name="xpool", bufs=1) as xpool,
            tc.tile_pool(name="acts", bufs=1) as acts,
            tc.tile_pool(name="inp", bufs=3) as inp,
            tc.tile_pool(name="tmp", bufs=3) as tmpp,
            tc.tile_pool(name="lay", bufs=6, space="PSUM") as lay,
            tc.tile_pool(name="accp", bufs=2, space="PSUM") as accp,
        ):
            # ---- persistent constants ----
            wsb = {}
            for key, dram in wts.items():
                t = wpool.tile(list(dram.shape), f32r, tag="w_" + key)
                nc.sync.dma_start(t[:], dram[:])
                wsb[key] = t
            bias_sb = wpool.tile([128, 25], f32, tag="biases")
            nc.sync.dma_start(bias_sb[:], bias_d[:])
            b4t_sb = wpool.tile([32, 64], f32r, tag="b4dfT")
            nc.sync.dma_start(b4t_sb[:], b4t_d[:])
            tsel_sb = wpool.tile([128, 124], f32r, tag="tsel")
            nc.sync.dma_start(tsel_sb[:], tsel_d[:])

            xt = []
            for j in range(unroll):
                t = xpool.tile([HID, BL], f32r, tag=f"x{j}")
                xt.append(t)
            nc.sync.dma_start(xt[0][:], x0_d[:])

            def mlp_l1(pfx, nei_t, x_in, bias_base, tag_sfx):
                """L1 of one MLP. nei-part matmuls are emitted first: they
                don't depend on x_in, so the PE chews on them while the
                previous step's x_next finalizes on DVE (keeps the PE busy
                across the recurrence tail, avoiding HAM re-throttle)."""
                h1 = acts.tile([128, 2048], f32r, tag="h1" + tag_sfx)
                w1x, w1n = wsb[pfx + "1x"], wsb[pfx + "1n"]
                ps = []
                for m in range(4):
                    p = lay.tile([128, 512], f32, tag="lay")
                    nc.tensor.matmul(p[:], r(w1n[:, m * 128:(m + 1) * 128]),
                                     r(nei_t[:]), start=True, stop=False)
                    ps.append(p)
                for m in range(4):
                    p = ps[m]
                    nc.tensor.matmul(p[:], r(w1x[:, m * 128:(m + 1) * 128]),
                                     r(x_in[:]), start=False, stop=True)
                    nc.scalar.activation(h1[:, m * 512:(m + 1) * 512], p[:], SILU,
                                         bias=bias_sb[:, bias_base + m:bias_base + m + 1])
                return h1

            def mlp_l23(pfx, h1, bias_base, tag_sfx):
                h_prev = h1
                for li, wkey in ((1, pfx + "2"), (2, pfx + "3")):
                    h = acts.tile([128, 2048], f32r, tag=f"h{li + 1}{tag_sfx}")
                    w = wsb[wkey]
                    for m in range(4):
                        p = lay.tile([128, 512], f32, tag="lay")
                        for k in range(4):
                            nc.tensor.matmul(
                                p[:],
                                r(w[:, k * 512 + m * 128: k * 512 + (m + 1) * 128]),
                                r(h_prev[:, k * 512:(k + 1) * 512]),
                                start=(k == 0), stop=(k == 3))
                        col = bias_base + 4 * li + m
                        nc.scalar.activation(h[:, m * 512:(m + 1) * 512], p[:], SILU,
                                             bias=bias_sb[:, col:col + 1])
                    h_prev = h
                return h_prev

            def step(s_expr, x_in, x_out):
                nei_t = inp.tile([65, BL], f32r, tag="nei")
                nc.sync.dma_start(nei_t[:], nei_d[ds(s_expr, 1)])
                dw4_t = inp.tile([128, BL], f32r, tag="dw4")
                nc.sync.dma_start(dw4_t[:], dw4_d[ds(s_expr, 1)])

                h1dr = mlp_l1("dr", nei_t, x_in, 0, "a")
                h1df = mlp_l1("df", nei_t, x_in, 12, "b")
                h3dr = mlp_l23("dr", h1dr, 0, "a")
                h3df = mlp_l23("df", h1df, 12, "b")

                acc = accp.tile([HID, BL], f32, tag="acc")
                # drift L4 (dt and .909 folded into weights)
                w4dr = wsb["dr4"]
                for k in range(4):
                    nc.tensor.matmul(acc[:],
                                     r(w4dr[:, k * HID:(k + 1) * HID]),
                                     r(h3dr[:, k * 512:(k + 1) * 512]),
                                     start=(k == 0), stop=False)
                # diffusion bias term: b4df.reshape(64,32).T @ dW
                nc.tensor.matmul(acc[:], r(b4t_sb[:]), r(dw4_t[0:32, :]),
                                 start=False, stop=False)
                # diffusion L4 + einsum
                w4df = wsb["df4"]
                for c in range(16):
                    pd = lay.tile([128, 512], f32, tag="lay")
                    for k in range(4):
                        nc.tensor.matmul(
                            pd[:],
                            r(w4df[:, k * 2048 + c * 128: k * 2048 + (c + 1) * 128]),
                            r(h3df[:, k * 512:(k + 1) * 512]),
                            start=(k == 0), stop=(k == 3))
                    tmp = tmpp.tile([128, 512], f32r, tag="tmp")
                    nc.vector.tensor_mul(tmp[:], pd[:], dw4_t[:].bitcast(f32))
                    nc.tensor.matmul(acc[:],
                                     r(tsel_sb[:, 60 - 4 * c: 124 - 4 * c]),
                                     r(tmp[:]), start=False, stop=(c == 15))
                # x_next = (acc + dt*b4dr) + x_in
                nc.vector.scalar_tensor_tensor(
                    x_out[:], acc[:], bias_sb[0:HID, 24:25],
                    x_in[:].bitcast(f32), op0=ADD, op1=ADD)
                nc.sync.dma_start(out_d[ds(s_expr, 1)], x_out[:].bitcast(f32))

            with tc.For_i(0, nsteps, unroll,
                          hint_engines=(mybir.EngineType.PE,)) as iv:
                for u in range(unroll):
                    step(iv + u, xt[u], xt[(u + 1) % unroll])

    _split_excess_waits(nc, mybir)
    return nc


def _pack_k(W):
    """[K, M] -> [128, (K/128)*M] with K-chunk k at columns [k*M, (k+1)*M)."""
    K, M = W.shape
    assert K % 128 == 0
    return np.concatenate([W[k * 128:(k + 1) * 128] for k in range(K // 128)],
                          axis=1)


def _prep_inputs(ts, nei, initial, bm, drift_params, diffusion_params,
                 nsteps=NSTEPS):
    ts = np.asarray(ts, np.float32)
    nei = np.asarray(nei, np.float32)
    initial = np.asarray(initial, np.float32)
    bm = np.asarray(bm, np.float32)
    dts = (ts[1:] - ts[:-1]).astype(np.float32)
    dt = float(dts.mean())

    dr = [(np.asarray(w, np.float32), np.asarray(b, np.float32))
          for w, b in drift_params]
    df = [(np.asarray(w, np.float32), np.asarray(b, np.float32))
          for w, b in diffusion_params]

    common = {}
    for pfx, params in (("dr", dr), ("df", df)):
        W1 = params[0][0]
        common["w1x_" + pfx] = np.ascontiguousarray(W1[0:64])
        common["w1n_" + pfx] = np.ascontiguousarray(W1[64:129])
        common["w2_" + pfx] = _pack_k(LSW * params[1][0])
        common["w3_" + pfx] = _pack_k(LSW * params[2][0])
    common["w4_dr"] = _pack_k((LSW * dt) * dr[3][0])
    common["w4_df"] = _pack_k(LSW * df[3][0])

    biases = np.zeros((128, 25), np.float32)
    for j, (pfx, params) in enumerate((("dr", dr), ("df", df))):
        for li in range(3):
            b = params[li][1]
            for m in range(4):
                biases[:, 12 * j + 4 * li + m] = b[m * 128:(m + 1) * 128]
    biases[0:HID, 24] = dt * dr[3][1]
    common["biases"] = biases
    common["b4dfT"] = np.ascontiguousarray(df[3][1].reshape(HID, NOISE).T)

    tsel = np.zeros((128, 124), np.float32)
    for p in range(128):
        tsel[p, 60 + p // 32] = 1.0
    common["tsel"] = tsel

    sq = np.sqrt(dts[:nsteps]).astype(np.float32)[:, None, None]
    in_maps = []
    for j in range(NCORES):
        sl = slice(j * BL, (j + 1) * BL)
        m = dict(common)
        m["nei"] = np.ascontiguousarray(
            nei[sl, :nsteps].transpose(1, 2, 0))                  # [S, 65, BL]
        bmt = (bm[:nsteps, sl] * sq).transpose(0, 2, 1)           # [S, 32, BL]
        m["dw4"] = np.ascontiguousarray(np.tile(bmt, (1, 4, 1)))  # [S, 128, BL]
        m["x0"] = np.ascontiguousarray(initial[sl].T)             # [64, BL]
        in_maps.append(m)
    return in_maps


def _run(inputs, trace=False, nsteps=NSTEPS, unroll=UNROLL):
    from concourse.bass_utils import run_bass_kernel_spmd

    key = (nsteps, unroll)
    if key not in _CACHE:
        _CACHE[key] = _build_nc(nsteps, unroll)
    nc = _CACHE[key]

    in_maps = _prep_inputs(
        inputs["ts"], inputs["nei"], inputs["initial"], inputs["bm"],
        inputs["drift_params"], inputs["diffusion_params"], nsteps)

    res = run_bass_kernel_spmd(nc, in_maps, core_ids=list(range(NCORES)),
                               trace=trace)

    ts = np.asarray(inputs["ts"], np.float32)
    initial = np.asarray(inputs["initial"], np.float32)
    L = ts.shape[0]
    out = np.empty((BATCH, L, 1 + HID), np.float32)
    out[:, :, 0] = ts[None, :]
    out[:, 0, 1:] = initial
    for j in range(NCORES):
        sl = slice(j * BL, (j + 1) * BL)
        xs = res.results[j]["out"]                     # [S, 64, BL]
        out[sl, 1:nsteps + 1, 1:] = xs.transpose(2, 0, 1)
    return out, res


def kernel(ts, nei, initial, bm, drift_params, diffusion_params,
           batch_size=None):
    out, _ = _run(dict(ts=ts, nei=nei, initial=initial, bm=bm,
                       drift_params=drift_params,
                       diffusion_params=diffusion_params))
    return out
